# revision 95
# baseline (speedup 1.0000x reference)
"""Trainium2 Bass kernel for nn_Dense: y = gelu_tanh(fp8qdq(x) @ fp8qdq(W) + b).

Strategy
--------
Host side: quantize x and W to float8_e4m3fn exactly as the reference does
(scale=1 quantize/dequantize), pre-interleave both operands CHUNK-major
([chunk, partition, ks, inner]) so every input DMA reads its 128
per-partition rows from ADJACENT DRAM — partition-major layout (1KB reads
at 8KB stride) throttled the early HBM supply rate and cost ~1.1-1.7us
(the biggest single win found).  Shard 2-D: 4 token-shards x 2
unit-shards across the 8 cores (minimizes per-core input bytes: 1MB x +
2MB W fp8 vs 4.5MB pure data-parallel; 24MB device-wide is the minimum
for any 2-D grid).

The device writes y as bfloat16 (upcast to f32 on the host): gelu outputs
round-trip through bf16 at ~1.1e-3 norm rel err (vs the 2e-2 gate).  The
128 DoubleRow matmuls hold the PE at its 216ns/matmul fp8 peak (~27.6us),
the binding floor; measured exec ~45-46us = ~6us NEFF entry + ~5us
supply-gated ramp + 27.6us PE + ~2us tail + ~2.3us fixed exit epilogue.

Device side (per core), hand-rolled semaphore pipeline (no TileContext —
saves the tile entry/exit barriers).  Only sync and scalar have HWDGE
queues:
  sync   : w0 ks0-3 leads (this queue's data starts ~0.8us before
           scalar's, so the big gate rides it), w0 ks4-7, xt2, xt6+7
           merged, w3; then the odd-mi merged output units
  scalar : xt0 leads, xt1, xt3, xt4+5 merged, w1, w2 in need-order; then
           per group: Gelu_apprx_tanh PSUM->SBUF (f32->bf16) and the
           even-mi merged output units
  tensor : 5 big + 20 short dummy DoubleRow matmuls warm the PE clock
           while inputs are in flight (the short ones bridge right up to
           the supply gate), then g0 fully (w0a+xt0 gate kp0, w0bc gates
           kp2 — w0bc lands BEFORE xt1 so sequential beats interleaving
           g0/g1 halves), then g1 (xt1 gate), then groups 2..29
           (column-major, 4 DoubleRow fp8 matmuls K=256 each into 7
           rotating PSUM banks), then the last two groups column-split
           into 256-wide half-groups so their gelus/DMAs overlap the
           final matmuls.
Outputs are merged per (row, column-pair): gelu of (mi, ni) fills slot
ni%2 of a [P, 2, NT] pair-buffer; one 256KB DMA ships both columns when
the pair completes.  This halves output triggers and defers all output
traffic past the early input crunch (which is supply-bound at only
~100-150GB/s/queue).  The last rows' tiles drain 2-wide across both
queues.

Hard-won constraints (verified empirically on hardware):
  * Do NOT split one logical input chunk into several small (<1KB per
    partition) DMAs with separate completion semaphores — the completion
    increments can fire before adjacent-split data is visible and the PE
    reads stale SBUF.  Merging adjacent chunks under ONE DMA+sem is safe.
  * Each dma_start trigger costs ~600-700ns of engine issue time and the
    first trigger cannot issue before ~7.2us (entry barrier + iram
    fetch), so fewer, bigger input DMAs get the stream queued sooner.
    Splitting w0 into four 1KB/partition per-kp gates was tried and is
    WORSE: the extra triggers starve xt1/xt2 (+1.2us).
  * Keep the per-queue input list in global need-order.  Moving xt1 to
    sync between w0a and w0bc starves the w0bc gate (+4us!).  Merging
    xt0+xt1 into one 262KB DMA is a wash (A/B'd both orders).
  * swap_lead (xt0 leading sync, xt1 leading scalar, g1 started first)
    produces NONDETERMINISTIC garbage (NaN / rel err 0.06 on some runs)
    — the sem-fires-before-data-visible race hits when the very first
    consumer reads a chunk right at its completion gate on both queues.
    DO NOT re-enable without a hardware-level fix.
  * Matmul PSUM outputs must start at a PSUM bank boundary: a chain into
    ps[:, 256:512] crashes the PE (INTERNAL/engine fault).  The tail
    half-groups therefore each take a full bank at offset 0 (ps_warm for
    one, rotation banks for the rest), which also avoids the act-reads-
    bank-while-PE-accumulates-same-bank hazard.
  * gpsimd software-DGE DMA as a third input stream was tried (xt45+xt67)
    and is ~3us WORSE — SWDGE is slow and disrupts the HW queues.
  * The dummy warm-up activation of the old design is NET NEGATIVE
    (A/B'd): its differently-typed table load costs scalar time and the
    real (psum f32 -> bf16) gelu table load before gelu#1 is not on the
    critical path.  No dummy activation.
  * The gpsimd end-of-run dma_reset/sem_clear is unnecessary — the
    runtime re-inits semaphores between executions (verified by checking
    output correctness of repeated profiled executions).
  * The PE runs at HALF clock (427ns/matmul) until the HW's
    high-activity clock boost engages, roughly after ~7us of cumulative
    PE busy time (cold).  Engagement is device-state dependent (often
    earlier when recently active) — this is the main run-to-run variance
    (+-1.5us).  The warmup matmuls both bridge to the supply gate AND
    accumulate busy time toward the boost.
  * Device-state drift (a slow-clock mode where EVERYTHING runs ~15-20%
    longer for a minute or so) makes single-batch comparisons
    meaningless: A/B variants interleaved in one process (see ab.py).

The fp8 products are exact in f32 accumulation, so the deviation from the
f32 reference is summation order + the gelu LUT + the bf16 output
rounding (~1.7e-3 norm rel err total).

TRN's e4m3 (ml_dtypes.float8_e4m3, IEEE-ish, max 240) and the reference's
float8_e4m3fn (OCP, max 448) share bit patterns for |v| <= 240; inputs here
are |v| < ~16 so a byte-level reinterpret is exact.

bias is zero in this problem's setup_inputs; a general Tile-based path with
a broadcast bias add is kept for nonzero bias (f32 output on that path).
"""

import sys

sys.path.insert(0, "/opt/trn_rl_repo")

from contextlib import ExitStack

import ml_dtypes
import numpy as np

import concourse.bacc as bacc
import concourse.mybir as mybir
from concourse.bass_utils import run_bass_kernel_spmd

N_CORES = 8
TOKENS, D_IN, UNITS = 4096, 1024, 4096

TOK_GRID, UNIT_GRID = 4, 2
TOK_SH = TOKENS // TOK_GRID          # 1024
UNIT_SH = UNITS // UNIT_GRID         # 2048

P = 128
KS = D_IN // P                       # 8 k-subtiles of 128
KP = KS // 2                         # 4 DoubleRow k-pairs (K=256 each)
M_TILES = TOK_SH // P                # 8
NT = 512                             # one PSUM bank of f32
N_TILES = UNIT_SH // NT              # 4
GROUPS = M_TILES * N_TILES           # 32
# Column m-tile processing order.  An arrival-order permutation
# ([0,1,3,2,...]) was tried and measured NEUTRAL-to-worse: the 3rd/4th xt
# chunks arrive ~15.5us regardless of which queue carries them (early
# supply-curve bound), so reordering just renames the binding waiter.
MI_SEQ = list(range(M_TILES))


def _g_to_tile(g):
    ni, slot = divmod(g, M_TILES)
    return ni, MI_SEQ[slot]

NB = 7                               # PSUM banks in rotation
OB = GROUPS                          # one SBUF output slot per group (no reuse)
RESET_SEMS = False                   # gpsimd end-of-run sem reset (see below)
N_WARM_BIG = 5                       # 1024-row dummy matmuls (PE clock warm)
N_WARM_SMALL = 10                    # short trailing dummies: fine-grained
                                     # handoff to the first gated matmul,
                                     # long enough to bridge the input wait
                                     # without a clock-gate dip

_prog_cache = {}

# Tunables (A/B-tested; see _build_raw_program).
DEFAULT_OPTS = (
    ("dummy_act", "none"),  # 'pre' triggers | 'post' triggers | 'none'
    ("warm_small", 20),
    ("tail_split", 2),      # how many trailing groups run column-split (0/2/4)
    ("xt1_on_sync", False),  # xt1 rides sync (between w0a and w0bc)
    ("xt01_merge", False),   # xt0+xt1 as one 262KB DMA on scalar
    ("swap_lead", False),    # xt0 leads sync, xt1 leads scalar
    ("prologue", "seq"),     # 'inter' g0/g1 halves interleaved | 'seq'
    ("warm_pre", 0),         # tiny matmuls BEFORE the big warmups (earlier
                             # PE activity -> earlier clock boost)
    ("tail_q4", True),       # quarter-split the very last half-group
    ("dense_layout", True),   # chunk-major DRAM layout (dense partition
                              # reads) instead of partition-major
    ("warm_gap", 0),          # deliberate PE-array idle (N quick sem
                              # re-checks) between warmups and g0 — lets
                              # the clock boost switch during idle
)


def _build_raw_program(opts_key=DEFAULT_OPTS):
    """Fast path (zero bias): raw bacc, hand-rolled semaphores."""
    opts = dict(opts_key)
    dummy_act = opts["dummy_act"]
    n_warm_small = opts["warm_small"]
    tail_n = int(opts["tail_split"])
    xt1_on_sync = opts["xt1_on_sync"]
    xt01_merge = opts["xt01_merge"]
    swap_lead = opts["swap_lead"]
    assert not (xt01_merge and xt1_on_sync)
    assert not (swap_lead and (xt01_merge or xt1_on_sync))
    assert tail_n in (0, 2, 4)
    n_main = GROUPS - tail_n
    # Tail half-group PSUM banks: h0 in the group's own rotation bank, h1 in
    # ps_warm (first tail group) or the top rotation banks — every chain at a
    # bank boundary (non-zero PSUM offsets for matmul output crash the PE),
    # and no act may read a bank while the PE accumulates into it.
    tail_h1_bank = {
        (GROUPS - tail_n + i): (None if i == 0 else NB - tail_n + i)
        for i in range(tail_n)
    }

    tail_q4 = opts["tail_q4"]
    assert not (tail_q4 and tail_n != 2)

    def tail_segments(g):
        # (col_start, col_width, psum_bank) chains for tail group g; every
        # chain starts at a bank boundary.  With tail_q4 the final group's
        # second half splits into two 128-col quarters (banks 5/6 are free
        # late: last main-loop writers g26/g27).
        hs = NT // 2
        own = psum[g % NB]
        h1 = ps_warm if tail_h1_bank[g] is None else psum[tail_h1_bank[g]]
        if tail_q4 and g == GROUPS - 1:
            q = NT // 4
            return [(0, hs, own), (hs, q, h1), (hs + q, q, psum[NB - 2])]
        return [(0, hs, own), (hs, hs, h1)]

    def tail_gate(g):
        # gelu count that frees every bank tail group g writes (main-loop
        # acts are numbered g'+1).
        gate = g - NB + 1
        for _, _, bank in tail_segments(g):
            for b in range(NB):
                if bank is psum[b]:
                    last = max(
                        (gg for gg in range(n_main) if gg % NB == b),
                        default=None,
                    )
                    if last is not None:
                        gate = max(gate, last + 1)
        return gate

    def n_segs(g):
        return len(tail_segments(g)) if g >= n_main else 1

    def act_at(g, si):
        # gelu count after segment si of tail group g completes
        return n_main + sum(n_segs(gg) for gg in range(n_main, g)) + si + 1

    def act_done(g):
        # gelu count after which group g's output tile is fully written
        if g < n_main:
            return g + 1
        return act_at(g, n_segs(g) - 1)

    dense = opts["dense_layout"]
    nc = bacc.Bacc("TRN2", target_bir_lowering=False)

    if dense:
        xt_d = nc.dram_tensor(
            "xt", [M_TILES, P, KS, P], mybir.dt.float8e4, kind="ExternalInput"
        )
        w_d = nc.dram_tensor(
            "w", [N_TILES, P, KS, NT], mybir.dt.float8e4, kind="ExternalInput"
        )

        def xt_src(sl_mi, sl_ks=slice(None)):
            if isinstance(sl_mi, int):
                return xt_d[sl_mi, :, sl_ks, :]
            return xt_d[sl_mi, :, sl_ks, :].rearrange("c p k m -> p c k m")

        def w_src(ni, sl_ks=slice(None)):
            return w_d[ni, :, sl_ks, :]
    else:
        xt_p = nc.dram_tensor(
            "xt", [P, M_TILES, KS, P], mybir.dt.float8e4, kind="ExternalInput"
        )
        w_p = nc.dram_tensor(
            "w", [P, N_TILES, KS, NT], mybir.dt.float8e4, kind="ExternalInput"
        )

        def xt_src(sl_mi, sl_ks=slice(None)):
            return xt_p[:, sl_mi, sl_ks, :]

        def w_src(ni, sl_ks=slice(None)):
            return w_p[:, ni, sl_ks, :]
    y = nc.dram_tensor(
        "y", [TOK_SH, UNIT_SH], mybir.dt.bfloat16, kind="ExternalOutput"
    )

    xt_sb = nc.alloc_sbuf_tensor("xt_sb", [P, M_TILES, KS, P], mybir.dt.float8e4)
    w_sb = nc.alloc_sbuf_tensor("w_sb", [P, N_TILES, KS, NT], mybir.dt.float8e4)
    # One [P, 2, NT] pair-buffer per (mi, column-pair): gelu of (mi, ni)
    # fills slot ni%2; one merged 256KB DMA ships both columns.  This halves
    # output trigger count and defers output traffic past the input crunch.
    out_sb = [
        nc.alloc_sbuf_tensor(f"out_sb{i}", [P, 2, NT], mybir.dt.bfloat16)
        for i in range(2 * M_TILES)
    ]
    warm_sb = nc.alloc_sbuf_tensor("warm_sb", [P, 2, NT], mybir.dt.float8e4)
    if dummy_act != "none":
        scratch = nc.alloc_sbuf_tensor("scratch", [P, 8], mybir.dt.float32)
    psum = [
        nc.alloc_psum_tensor(f"ps{b}", [P, NT], mybir.dt.float32) for b in range(NB)
    ]
    ps_warm = nc.alloc_psum_tensor("ps_warm", [P, NT], mybir.dt.float32)

    # NOTE: do NOT split one logical input chunk into multiple small DMAs
    # with separate completion semaphores — empirically (v4/v7) the HWDGE
    # completion increments can fire before adjacent-split data is fully
    # visible, racing the PE.  Whole-chunk DMAs are reliable, and MERGING
    # adjacent chunks under one DMA+sem is safe.  Each dma_start trigger
    # costs ~600-700ns of engine issue time (measured), so fewer, bigger
    # input DMAs get the whole stream into the queues sooner.
    xt0_sem = nc.alloc_semaphore("xt0_sem")     # xt mi 0
    xt1_sem = nc.alloc_semaphore("xt1_sem")     # xt mi 1
    xt2_sem = nc.alloc_semaphore("xt2_sem")     # xt mi 2
    xt3_sem = nc.alloc_semaphore("xt3_sem")     # xt mi 3
    xt45_sem = nc.alloc_semaphore("xt45_sem")   # xt mi 4-5 (merged)
    xt67_sem = nc.alloc_semaphore("xt67_sem")   # xt mi 6-7 (merged)
    w_sems = [nc.alloc_semaphore(f"w_sem{i}") for i in range(1, N_TILES)]
    w0a_sem = nc.alloc_semaphore("w0a_sem")     # w0 ks 0-3 (kp 0-1)
    w0bc_sem = nc.alloc_semaphore("w0bc_sem")   # w0 ks 4-7 (kp 2-3)
    mm_sem = nc.alloc_semaphore("mm_sem")
    gelu_sem = nc.alloc_semaphore("gelu_sem")
    out_semA = nc.alloc_semaphore("out_semA")   # scalar-queue outputs (even g)
    out_semB = nc.alloc_semaphore("out_semB")   # sync-queue outputs (odd g)
    all_sems = [xt0_sem, xt1_sem, xt2_sem, xt3_sem, xt45_sem, xt67_sem] + w_sems + [
        w0a_sem, w0bc_sem, mm_sem, gelu_sem, out_semA, out_semB
    ]
    # first group of column 0 that must wait on each xt sem
    xt_gate = {
        0: xt0_sem, 1: xt1_sem, 2: xt2_sem, 3: xt3_sem, 4: xt45_sem, 6: xt67_sem
    }

    # no_gpsimd_drain: skip the compiler's end-of-block GpSimd dge_drain —
    # the gpsimd section below already dma_reset()s (drains) the kernel sem
    # range, so the extra drain only lengthens the exit barrier.
    with nc.Block(no_gpsimd_drain=True) as block:

        @block.sync
        def _(sync):
            # Inputs in per-queue deadline order; the two queues drain at a
            # similar rate, so cumulative-bytes-before-chunk on each queue
            # is matched against each gate's deadline.  (A tiny leading
            # "pump" DMA per queue was tried and HURT: trigger issue time
            # ~600-700ns each delays the real stream.)
            # w0 first-half leads sync (this queue's data starts ~0.8us
            # before the scalar queue's, so the big w0a gate rides it while
            # xt0 — smaller — rides scalar).  With swap_lead, xt0 leads sync
            # and xt1 leads scalar so g0/g1 gates open from separate queues.
            if swap_lead:
                sync.dma_start(out=xt_sb[:, 0, :, :], in_=xt_src(0)).then_inc(
                    xt0_sem, 16
                )
            sync.dma_start(out=w_sb[:, 0, 0:4, :], in_=w_src(0, slice(0, 4))).then_inc(
                w0a_sem, 16
            )
            if xt1_on_sync:
                sync.dma_start(out=xt_sb[:, 1, :, :], in_=xt_src(1)).then_inc(
                    xt1_sem, 16
                )
            sync.dma_start(out=w_sb[:, 0, 4:KS, :], in_=w_src(0, slice(4, KS))).then_inc(
                w0bc_sem, 16
            )
            sync.dma_start(out=xt_sb[:, 2, :, :], in_=xt_src(2)).then_inc(
                xt2_sem, 16
            )
            sync.dma_start(
                out=xt_sb[:, 6:M_TILES, :, :], in_=xt_src(slice(6, M_TILES))
            ).then_inc(xt67_sem, 16)
            sync.dma_start(out=w_sb[:, 3, :, :], in_=w_src(3)).then_inc(
                w_sems[2], 16
            )
            # Merged output units (one [P, 2*NT] DMA per odd-mi row/pair),
            # gated on the pair's second gelu — defers output traffic past
            # the input crunch.  mi=7's ni=2 tile goes single; the last two
            # rows' ni=3 tiles drain 2-wide across both queues.
            for mi in (1, 3, 5, 7):
                sync.wait_ge(gelu_sem, M_TILES + mi + 1)
                sync.dma_start(
                    out=y[mi * P : (mi + 1) * P, 0 : 2 * NT],
                    in_=out_sb[mi][:, :, :],
                ).then_inc(out_semB, 16)
            sync.wait_ge(gelu_sem, 24)
            sync.dma_start(
                out=y[7 * P : 8 * P, 2 * NT : 3 * NT],
                in_=out_sb[M_TILES + 7][:, 0, :],
            ).then_inc(out_semB, 16)
            for mi in (1, 3, 5):
                sync.wait_ge(gelu_sem, act_done(3 * M_TILES + mi))
                sync.dma_start(
                    out=y[mi * P : (mi + 1) * P, 2 * NT : 4 * NT],
                    in_=out_sb[M_TILES + mi][:, :, :],
                ).then_inc(out_semB, 16)
            sync.wait_ge(gelu_sem, act_done(3 * M_TILES + 6))
            sync.dma_start(
                out=y[6 * P : 7 * P, 3 * NT + NT // 2 : 4 * NT],
                in_=out_sb[M_TILES + 6][:, 1, NT // 2 : NT],
            ).then_inc(out_semB, 16)
            # mi7 (the final group): one DMA per tail segment past the first
            # (which scalar ships) — with tail_q4 the last two ship as 32KB
            # quarters pipelined behind each quarter-gelu.
            gl = GROUPS - 1
            if tail_n == 0:
                n_outB = 10
                sync.wait_ge(gelu_sem, act_done(gl))
                sync.dma_start(
                    out=y[7 * P : 8 * P, 3 * NT + NT // 2 : 4 * NT],
                    in_=out_sb[M_TILES + 7][:, 1, NT // 2 : NT],
                ).then_inc(out_semB, 16)
            else:
                segs_l = tail_segments(gl)
                n_outB = 9 + len(segs_l) - 1
                for si in range(1, len(segs_l)):
                    c0, cw, _ = segs_l[si]
                    sync.wait_ge(gelu_sem, act_at(gl, si))
                    sync.dma_start(
                        out=y[7 * P : 8 * P, 3 * NT + c0 : 3 * NT + c0 + cw],
                        in_=out_sb[M_TILES + 7][:, 1, c0 : c0 + cw],
                    ).then_inc(out_semB, 16)
            sync.wait_ge(out_semB, 16 * n_outB)

        @block.tensor
        def _(t):
            # Warm the HAM clock gate while input DMAs are in flight: big
            # dummies first, then short ones so the engine can slip into the
            # first real (gated) matmul with fine granularity.
            for _i in range(opts["warm_pre"]):
                t.matmul(
                    ps_warm[:, 0:128],
                    lhsT=warm_sb[:, :, 0:P],
                    rhs=warm_sb[:, :, 0:128],
                    start=True,
                    stop=True,
                    perf_mode=mybir.MatmulPerfMode.DoubleRow,
                )
            for _i in range(N_WARM_BIG):
                t.matmul(
                    ps_warm[:, :],
                    lhsT=warm_sb[:, :, 0:P],
                    rhs=warm_sb[:, :, :],
                    start=True,
                    stop=True,
                    perf_mode=mybir.MatmulPerfMode.DoubleRow,
                )
            for _i in range(n_warm_small):
                t.matmul(
                    ps_warm[:, 0:128],
                    lhsT=warm_sb[:, :, 0:P],
                    rhs=warm_sb[:, :, 0:128],
                    start=True,
                    stop=True,
                    perf_mode=mybir.MatmulPerfMode.DoubleRow,
                )
            def mm_run(g, kps, inc=False):
                ni, mi = _g_to_tile(g)
                ps = psum[g % NB]
                for kp in kps:
                    mm = t.matmul(
                        ps[:, :],
                        lhsT=xt_sb[:, mi, 2 * kp : 2 * kp + 2, :],
                        rhs=w_sb[:, ni, 2 * kp : 2 * kp + 2, :],
                        start=(kp == 0),
                        stop=(kp == KP - 1),
                        perf_mode=mybir.MatmulPerfMode.DoubleRow,
                    )
                if inc:
                    mm.then_inc(mm_sem)

            # Interleaved prologue: g0 kp0-1 starts on the minimal leading
            # bytes (w0a + xt0); g1 kp0-1 (xt1) runs while w0 ks4-7 is still
            # in flight, then both second halves — hides the w0bc transfer
            # behind real matmuls.
            for _ in range(opts["warm_gap"]):
                t.wait_ge(w0a_sem, 16)
            if opts["prologue"] == "seq":
                # w0bc (sync 2nd, cum 512K) now lands BEFORE xt1 (scalar
                # 2nd): run g0 fully, then g1 — the xt1 wait absorbs into
                # g0's second half.
                t.wait_ge(w0a_sem, 16)
                t.wait_ge(xt0_sem, 16)
                mm_run(0, (0, 1))
                t.wait_ge(w0bc_sem, 16)
                mm_run(0, (2, 3), inc=True)
                t.wait_ge(xt1_sem, 16)
                mm_run(1, range(KP), inc=True)
            elif swap_lead:
                # xt1 (scalar-lead) usually lands first: start g1.
                t.wait_ge(xt1_sem, 16)
                mm_run(1, (0, 1))
                t.wait_ge(w0a_sem, 16)
                t.wait_ge(xt0_sem, 16)
                mm_run(0, (0, 1))
                t.wait_ge(w0bc_sem, 16)
                mm_run(0, (2, 3), inc=True)
                mm_run(1, (2, 3), inc=True)
            else:
                t.wait_ge(w0a_sem, 16)
                t.wait_ge(xt0_sem, 16)
                mm_run(0, (0, 1))
                if not xt01_merge:
                    t.wait_ge(xt1_sem, 16)
                mm_run(1, (0, 1))
                t.wait_ge(w0bc_sem, 16)
                mm_run(0, (2, 3), inc=True)
                mm_run(1, (2, 3), inc=True)
            for g in range(2, n_main):
                ni, mi = _g_to_tile(g)
                if mi == 0 and ni > 0:
                    t.wait_ge(w_sems[ni - 1], 16)
                if ni == 0 and mi in xt_gate:
                    t.wait_ge(xt_gate[mi], 16)
                if g >= NB:
                    t.wait_ge(gelu_sem, g - NB + 1)
                mm_run(g, range(KP), inc=True)
            # Trailing groups run column-split (two 256-wide half-groups
            # each) so their gelus and output DMAs overlap the final
            # matmuls instead of serializing after them.
            for g in range(n_main, GROUPS):
                ni, mi = _g_to_tile(g)
                t.wait_ge(gelu_sem, tail_gate(g))
                for c0, cw, bank in tail_segments(g):
                    for kp in range(KP):
                        mm = t.matmul(
                            bank[:, 0:cw],
                            lhsT=xt_sb[:, mi, 2 * kp : 2 * kp + 2, :],
                            rhs=w_sb[:, ni, 2 * kp : 2 * kp + 2, c0 : c0 + cw],
                            start=(kp == 0),
                            stop=(kp == KP - 1),
                            perf_mode=mybir.MatmulPerfMode.DoubleRow,
                        )
                    mm.then_inc(mm_sem)

        @block.scalar
        def _(s):
            def emit_dummy_act():
                # Dummy activation: hoists the gelu act-table load to the
                # front of the scalar stream so it overlaps the input DMAs
                # instead of landing right before the first real gelu.
                s.activation(
                    scratch[:, :],
                    scratch[:, :],
                    mybir.ActivationFunctionType.Gelu_apprx_tanh,
                )

            if dummy_act == "pre":
                emit_dummy_act()
            # xt0 leads this queue (group 0's other gate); the odd-mi xt
            # chunks and w1/w2 follow in need-order.
            if swap_lead:
                s.dma_start(out=xt_sb[:, 1, :, :], in_=xt_src(1)).then_inc(
                    xt1_sem, 16
                )
            elif xt01_merge:
                s.dma_start(out=xt_sb[:, 0:2, :, :], in_=xt_src(slice(0, 2))).then_inc(
                    xt0_sem, 16
                )
            else:
                s.dma_start(out=xt_sb[:, 0, :, :], in_=xt_src(0)).then_inc(
                    xt0_sem, 16
                )
                if not xt1_on_sync:
                    s.dma_start(
                        out=xt_sb[:, 1, :, :], in_=xt_src(1)
                    ).then_inc(xt1_sem, 16)
            s.dma_start(out=xt_sb[:, 3, :, :], in_=xt_src(3)).then_inc(
                xt3_sem, 16
            )
            s.dma_start(out=xt_sb[:, 4:6, :, :], in_=xt_src(slice(4, 6))).then_inc(
                xt45_sem, 16
            )
            s.dma_start(out=w_sb[:, 1, :, :], in_=w_src(1)).then_inc(
                w_sems[0], 16
            )
            s.dma_start(out=w_sb[:, 2, :, :], in_=w_src(2)).then_inc(
                w_sems[1], 16
            )
            if dummy_act == "post":
                emit_dummy_act()
            for g in range(n_main):
                ni, mi = _g_to_tile(g)
                p, col = divmod(ni, 2)
                ob = out_sb[p * M_TILES + mi]
                s.wait_ge(mm_sem, g + 1)
                s.activation(
                    ob[:, col, :],
                    psum[g % NB][:, :],
                    mybir.ActivationFunctionType.Gelu_apprx_tanh,
                ).then_inc(gelu_sem)
                if col == 1 and mi % 2 == 0 and not (p == 1 and mi == 6):
                    s.dma_start(
                        out=y[mi * P : (mi + 1) * P, 2 * p * NT : (2 * p + 2) * NT],
                        in_=ob[:, :, :],
                    ).then_inc(out_semA, 16)
                elif g == 22:  # mi=6's ni=2 tile goes single (early)
                    s.dma_start(
                        out=y[6 * P : 7 * P, 2 * NT : 3 * NT],
                        in_=ob[:, 0, :],
                    ).then_inc(out_semA, 16)
                elif tail_n == 0 and g >= GROUPS - 2:
                    # first halves of the 2-wide tail
                    s.dma_start(
                        out=y[mi * P : (mi + 1) * P, 3 * NT : 3 * NT + NT // 2],
                        in_=ob[:, 1, 0 : NT // 2],
                    ).then_inc(out_semA, 16)
            # Gelu + ship each 256-wide half as soon as its PSUM half
            # closes.  Scalar ships: mi<=5 even rows' full pair after h1,
            # mi 6/7 h0 halves (sync takes the h1 halves).
            mmc = n_main
            for g in range(n_main, GROUPS):
                ni, mi = _g_to_tile(g)
                ob = out_sb[M_TILES + mi]
                segs = tail_segments(g)
                for si, (c0, cw, bank) in enumerate(segs):
                    mmc += 1
                    s.wait_ge(mm_sem, mmc)
                    s.activation(
                        ob[:, 1, c0 : c0 + cw],
                        bank[:, 0:cw],
                        mybir.ActivationFunctionType.Gelu_apprx_tanh,
                    ).then_inc(gelu_sem)
                    if si == 0 and mi >= 6:
                        s.dma_start(
                            out=y[
                                mi * P : (mi + 1) * P,
                                3 * NT : 3 * NT + NT // 2,
                            ],
                            in_=ob[:, 1, 0 : NT // 2],
                        ).then_inc(out_semA, 16)
                    elif si == len(segs) - 1 and mi % 2 == 0 and mi < 6:
                        s.dma_start(
                            out=y[mi * P : (mi + 1) * P, 2 * NT : 4 * NT],
                            in_=ob[:, :, :],
                        ).then_inc(out_semA, 16)
            s.wait_ge(out_semA, 16 * 10)

        @block.gpsimd
        def _(gp):
            # NOTE: carrying input chunks on gpsimd's software-DGE queue was
            # tried (xt45+xt67) and measured ~3us WORSE — SWDGE is too slow
            # and disrupts the HW queues.  Keep gpsimd idle.
            gp.nop()
            if RESET_SEMS:
                # Reset semaphores so repeat executions of the loaded NEFF
                # stay correct regardless of runtime re-init behavior.
                gp.wait_ge(out_semA, 16 * 10)
                gp.wait_ge(out_semB, 16 * 10)
                nums = sorted(sh.num for sh in all_sems)
                lo, hi = nums[0], nums[-1] + 1
                assert nums == list(range(lo, hi))
                gp.dma_reset(range(lo, hi))
                gp.sem_clear(range(lo, hi))

    nc.compile()
    return nc


def _build_tile_program():
    """General path (nonzero bias): TileContext with broadcast bias add."""
    import concourse.tile as tile

    nc = bacc.Bacc("TRN2", target_bir_lowering=False)

    xt = nc.dram_tensor("xt", [D_IN, TOK_SH], mybir.dt.float8e4, kind="ExternalInput")
    w = nc.dram_tensor("w", [D_IN, UNIT_SH], mybir.dt.float8e4, kind="ExternalInput")
    b = nc.dram_tensor("b", [1, UNIT_SH], mybir.dt.float32, kind="ExternalInput")
    y = nc.dram_tensor("y", [TOK_SH, UNIT_SH], mybir.dt.float32, kind="ExternalOutput")

    with tile.TileContext(nc) as tc, ExitStack() as ctx:
        xt_pool = ctx.enter_context(tc.tile_pool(name="xt", bufs=1))
        w_pool = ctx.enter_context(tc.tile_pool(name="w", bufs=1))
        out_pool = ctx.enter_context(tc.tile_pool(name="out", bufs=8))
        psum_pool = ctx.enter_context(tc.tile_pool(name="psum", bufs=6, space="PSUM"))
        bias_pool = ctx.enter_context(tc.tile_pool(name="bias", bufs=1))
        tmp_pool = ctx.enter_context(tc.tile_pool(name="tmp", bufs=4))

        xt_tile = xt_pool.tile([P, KS, TOK_SH], mybir.dt.float8e4)
        xt_re = xt[:, :].rearrange("(ks p) m -> p ks m", p=P)
        nc.sync.dma_start(xt_tile[:, :, 0:P], xt_re[:, :, 0:P])

        w_tiles = [
            w_pool.tile([P, KS, NT], mybir.dt.float8e4, name=f"w{ni}", tag=f"w{ni}")
            for ni in range(N_TILES)
        ]
        for ni in range(N_TILES):
            nc.sync.dma_start(
                w_tiles[ni][:, :, :],
                w[:, ni * NT : (ni + 1) * NT].rearrange("(ks p) n -> p ks n", p=P),
            )
        for mi in range(1, M_TILES):
            nc.sync.dma_start(
                xt_tile[:, :, mi * P : (mi + 1) * P],
                xt_re[:, :, mi * P : (mi + 1) * P],
            )

        bias_bcast = bias_pool.tile([P, UNIT_SH], mybir.dt.float32)
        nc.sync.dma_start(bias_bcast[:, :], b[0, :].partition_broadcast(P))

        for mi in range(M_TILES):
            for ni in range(N_TILES):
                ps = psum_pool.tile([P, NT], mybir.dt.float32)
                for kp in range(KP):
                    nc.tensor.matmul(
                        ps[:, :],
                        lhsT=xt_tile[:, 2 * kp : 2 * kp + 2, mi * P : (mi + 1) * P],
                        rhs=w_tiles[ni][:, 2 * kp : 2 * kp + 2, :],
                        start=(kp == 0),
                        stop=(kp == KP - 1),
                        perf_mode=mybir.MatmulPerfMode.DoubleRow,
                    )
                ot = out_pool.tile([P, NT], mybir.dt.float32)
                tmp = tmp_pool.tile([P, NT], mybir.dt.float32)
                nc.vector.tensor_add(
                    tmp[:, :], ps[:, :], bias_bcast[:, ni * NT : (ni + 1) * NT]
                )
                nc.scalar.activation(
                    ot[:, :],
                    tmp[:, :],
                    mybir.ActivationFunctionType.Gelu_apprx_tanh,
                )
                nc.sync.dma_start(
                    y[mi * P : (mi + 1) * P, ni * NT : (ni + 1) * NT], ot[:, :]
                )
    nc.compile()
    return nc


def _get_program(with_bias: bool, opts_key=DEFAULT_OPTS):
    key = (with_bias, opts_key)
    if key not in _prog_cache:
        _prog_cache[key] = (
            _build_tile_program() if with_bias else _build_raw_program(opts_key)
        )
    return _prog_cache[key]


def _quantize(x, kernel):
    # fp8 quantize on host with reference (OCP e4m3fn) semantics; bytes are
    # reinterpreted as the TRN-compatible ml_dtypes.float8_e4m3 later.
    xq = np.asarray(x, np.float32).astype(ml_dtypes.float8_e4m3fn)
    wq = np.asarray(kernel, np.float32).astype(ml_dtypes.float8_e4m3fn)
    return xq.view(np.uint8), wq.view(np.uint8)


def _run(x, kernel, bias, trace=False, opts_key=DEFAULT_OPTS):
    assert x.shape == (TOKENS, D_IN) and kernel.shape == (D_IN, UNITS)
    xq_bits, wq_bits = _quantize(x, kernel)
    bf = np.asarray(bias, np.float32).reshape(UNITS)
    with_bias = bool(np.any(bf != 0))
    nc = _get_program(with_bias, opts_key)

    in_maps = []
    for c in range(N_CORES):
        tg, ug = divmod(c, UNIT_GRID)
        xs = xq_bits[tg * TOK_SH : (tg + 1) * TOK_SH, :]       # [1024, 1024]
        ws = wq_bits[:, ug * UNIT_SH : (ug + 1) * UNIT_SH]     # [1024, 2048]
        if with_bias:
            in_map = {
                "xt": np.ascontiguousarray(xs.T).view(ml_dtypes.float8_e4m3),
                "w": np.ascontiguousarray(ws).view(ml_dtypes.float8_e4m3),
                "b": np.ascontiguousarray(
                    bf[ug * UNIT_SH : (ug + 1) * UNIT_SH].reshape(1, UNIT_SH)
                ),
            }
        elif dict(opts_key).get("dense_layout", False):
            # Chunk-major: each chunk's 128 partition rows are adjacent in
            # DRAM (dense reads).  xt_host[mi, p, ks, m] = X[mi*128+m, ks*128+p]
            xt_host = np.ascontiguousarray(
                xs.reshape(M_TILES, P, KS, P).transpose(0, 3, 2, 1)
            )
            # w_host[ni, p, ks, n] = W[ks*128+p, ni*512+n]
            w_host = np.ascontiguousarray(
                ws.reshape(KS, P, N_TILES, NT).transpose(2, 1, 0, 3)
            )
            in_map = {
                "xt": xt_host.view(ml_dtypes.float8_e4m3),
                "w": w_host.view(ml_dtypes.float8_e4m3),
            }
        else:
            # Pre-interleave into [partition, chunk, ks, inner] DMA layouts.
            # xt_host[p, mi, ks, m] = X[mi*128+m, ks*128+p]
            xt_host = np.ascontiguousarray(
                xs.reshape(M_TILES, P, KS, P).transpose(3, 0, 2, 1)
            )
            # w_host[p, ni, ks, n] = W[ks*128+p, ni*512+n]
            w_host = np.ascontiguousarray(
                ws.reshape(KS, P, N_TILES, NT).transpose(1, 2, 0, 3)
            )
            in_map = {
                "xt": xt_host.view(ml_dtypes.float8_e4m3),
                "w": w_host.view(ml_dtypes.float8_e4m3),
            }
        in_maps.append(in_map)

    res = run_bass_kernel_spmd(nc, in_maps, list(range(N_CORES)), trace=trace)

    out = np.empty((TOKENS, UNITS), np.float32)
    for c in range(N_CORES):
        tg, ug = divmod(c, UNIT_GRID)
        ys = np.asarray(res.results[c]["y"])
        if ys.dtype != np.float32:
            ys = ys.astype(np.float32)
        out[tg * TOK_SH : (tg + 1) * TOK_SH, ug * UNIT_SH : (ug + 1) * UNIT_SH] = ys
    return out, res


def kernel(x: np.ndarray, kernel: np.ndarray, bias: np.ndarray) -> np.ndarray:
    return _run(x, kernel, bias)[0]


def _ensure_ntff_hook():
    """The agent image's antenv lacks axon_hooks; shim it so trace=True works."""
    try:
        from antenv.axon_hooks import get_axon_ntff_profile_hook  # noqa: F401

        return
    except ImportError:
        pass
    import types

    import antenv

    mod = types.ModuleType("antenv.axon_hooks")
    mod._hook = None

    def set_axon_ntff_profile_hook(h):
        mod._hook = h

    def get_axon_ntff_profile_hook():
        return mod._hook

    mod.set_axon_ntff_profile_hook = set_axon_ntff_profile_hook
    mod.get_axon_ntff_profile_hook = get_axon_ntff_profile_hook
    sys.modules["antenv.axon_hooks"] = mod
    antenv.axon_hooks = mod
    if "/root/.axon_site" not in sys.path:
        sys.path.insert(0, "/root/.axon_site")
    from trn_agent_boot.trn_boot import _ntff_profile_via_ctypes

    set_axon_ntff_profile_hook(
        _ntff_profile_via_ctypes("/opt/axon/libaxon_pjrt.so")
    )


def profile_run(np_inputs, opts_key=DEFAULT_OPTS):
    """Run with NTFF tracing; returns (exec_time_ns, output)."""
    _ensure_ntff_hook()
    out, res = _run(
        np_inputs["x"],
        np_inputs["kernel"],
        np_inputs["bias"],
        trace=True,
        opts_key=opts_key,
    )
    return res.exec_time_ns, out

